# revision 11
# baseline (speedup 1.0000x reference)
"""BERT self-attention (flash-style) Trainium2 Bass kernel.

Full inputs -> full output. Shards data-parallel over batch: batch element i
runs on NeuronCore i (B == 8 == n_cores), no collectives.

Host-side prep (cheap numpy): transpose x / Wqkv / out_w into the e-major
layouts the TensorE needs (lhsT/rhs must both be contraction-major), fold the
1/sqrt(d) scale into the q block of Wqkv, turn the key-padding mask into an
additive exp bias (0 / -30000) and the query mask into a 0/1 multiplier.

On-chip per core (S=1024, E=768, H=12, D=64):
  phase 1: v = x @ Wv.T (natural layout, ones column per head appended)
           qT,kT = (Wq|k @ x.T)  (head dims on partitions)
  phase 2: per head pair (2p, 2p+1):
           scoresT[sk,sq] = kT.T-free matmul, 2 heads concurrently via 64-row
           PE tiling; expT = Exp(scoresT + key_bias) on ScalarE (no max
           subtraction -- scores are O(1) by construction); ctxT[d,sq]
           accumulated in PSUM with denominator from the ones column;
           normalize via VectorE reciprocal + GpSimd partition broadcast.
  phase 3: out = ctxT.T @ out_w.T, query mask as per-partition scalar, + bias.
"""

import sys

if "/opt/trn_rl_repo" not in sys.path:
    sys.path.insert(0, "/opt/trn_rl_repo")

import numpy as np
import ml_dtypes

import concourse.bass as bass
import concourse.bacc as bacc
import concourse.tile as tile
from concourse import mybir
from concourse.bass_utils import run_bass_kernel_spmd

B, S, E, H = 8, 1024, 768, 12
D = E // H            # 64
NP = 128              # SBUF/PSUM partitions
EC = E // NP          # 6 e-chunks (contraction chunks)
FC = 3 * E // NP      # 18 f-chunks of the fused qkv output
SC = S // NP          # 8 sequence chunks
NPAIR = H // 2        # 6 head pairs
BF16 = mybir.dt.bfloat16
F32 = mybir.dt.float32
EXP = mybir.ActivationFunctionType.Exp
MASK_NEG = -30000.0   # exp(x + MASK_NEG) == 0 for any realistic score x


def _body(tc, xt, wqk, bqk, wot, bo, kb, qm, out, with_bias, repeat=1):
    nc = tc.nc

    with tc.tile_pool(name="const", bufs=1) as const:
        # ---- persistent SBUF state -------------------------------------
        xt_sb = const.tile([NP, EC, S], BF16)      # x.T   [e, s]
        nc.sync.dma_start(out=xt_sb, in_=xt.rearrange("(c p) s -> p c s", p=NP))
        wq_sb = const.tile([NP, EC, 3 * E], BF16)  # Wqkv.T [e, f]
        nc.sync.dma_start(out=wq_sb, in_=wqk.rearrange("(c p) f -> p c f", p=NP))
        wo_sb = const.tile([NP, EC, E], BF16)      # out_w.T [e, eout]
        nc.sync.dma_start(out=wo_sb, in_=wot.rearrange("(c p) f -> p c f", p=NP))
        if with_bias:
            bq_sb = const.tile([NP, FC], F32)      # qkv bias, col j = f-chunk j
            nc.sync.dma_start(out=bq_sb, in_=bqk.rearrange("(c p) -> p c", p=NP))
            # v bias per head-dim column: col h = bias[2E + 64h + p], p in 0..63
            bvcol = const.tile([NP, H], F32)
            nc.sync.dma_start(
                out=bvcol[0:64, :],
                in_=bass.AP(tensor=bqk, offset=2 * E, ap=[[1, 64], [64, H]]),
            )
        kb_sb = const.tile([NP, SC], F32)          # key mask bias, col c = s-chunk c
        nc.sync.dma_start(out=kb_sb, in_=kb.rearrange("(c p) -> p c", p=NP))
        qm_sb = const.tile([NP, SC], F32)          # query mask 0/1, col m = s-chunk m
        nc.sync.dma_start(out=qm_sb, in_=qm.rearrange("(c p) -> p c", p=NP))
        if with_bias:
            bo_bc = const.tile([NP, E], F32)       # out bias broadcast
            nc.sync.dma_start(
                out=bo_bc, in_=bass.AP(tensor=bo, offset=0, ap=[[0, NP], [1, E]])
            )

        for _rep in range(repeat):
            _compute(tc, nc, with_bias,
                     xt_sb, wq_sb, wo_sb, kb_sb, qm_sb, out,
                     bq_sb if with_bias else None,
                     bvcol if with_bias else None,
                     bo_bc if with_bias else None)


def _compute(tc, nc, with_bias, xt_sb, wq_sb, wo_sb, kb_sb, qm_sb, out,
             bq_sb, bvcol, bo_bc):
    with tc.tile_pool(name="work", bufs=1) as work:
        # qT/kT: [128, j, s] bf16; partition = f within chunk. j=0..5 q pairs
        # (heads 2j,2j+1 at partitions 0-63 / 64-127), j=6..11 k pairs.
        qkT = work.tile([NP, H, S], BF16)
        # v (+ per-head denominator column): s-chunk m on partitions.
        # Head slot of 65 columns: [v(64), ones] -- the ones column makes the
        # ctx matmul emit the softmax denominator as psum row 64 for free.
        vsb = work.tile([NP, SC, H * 65], BF16)
        # ctx.T: pair j -> partitions 0:64 head 2j, 64:128 head 2j+1; e-chunk j.
        ctxT = work.tile([NP, EC, S], BF16)

        with tc.tile_pool(name="norm", bufs=4) as norm_pool, \
             tc.tile_pool(name="exps", bufs=18) as exps, \
             tc.tile_pool(name="osb", bufs=3) as outp:

            # ---- phase 1: fused qkv projection -------------------------
            with tc.tile_pool(name="ps_qkv", bufs=3, space="PSUM") as ps_qkv:
                # v natural: [s-chunk, f] so it can be ctx lhsT later
                for m in range(SC):
                    pv = ps_qkv.tile([NP, E], F32, tag="ps")
                    for k in range(EC):
                        st, sp = (k == 0), (k == EC - 1)
                        nc.tensor.matmul(
                            pv[:, 0:512],
                            lhsT=xt_sb[:, k, m * NP:(m + 1) * NP],
                            rhs=wq_sb[:, k, 2 * E:2 * E + 512],
                            start=st, stop=sp,
                        )
                        nc.tensor.matmul(
                            pv[:, 512:768],
                            lhsT=xt_sb[:, k, m * NP:(m + 1) * NP],
                            rhs=wq_sb[:, k, 2 * E + 512:3 * E],
                            start=st, stop=sp,
                        )
                    # per-head slot [v(64), ones]: the ones column makes the
                    # ctx matmul emit the softmax denominator as psum row 64
                    v_m = vsb[:, m, :].rearrange("p (h t) -> p h t", t=65)
                    nc.vector.memset(v_m[:, :, 64:65], 1.0)
                    pv_h = pv.rearrange("p (h d) -> p h d", d=D)
                    nc.vector.tensor_copy(out=v_m[:, :, 0:64], in_=pv_h)

                # qT, kT: f-chunk j on partitions, s free
                for j in range(H):
                    pq = ps_qkv.tile([NP, S], F32, tag="ps")
                    for k in range(EC):
                        st, sp = (k == 0), (k == EC - 1)
                        for n in (0, 512):
                            nc.tensor.matmul(
                                pq[:, n:n + 512],
                                lhsT=wq_sb[:, k, j * NP:(j + 1) * NP],
                                rhs=xt_sb[:, k, n:n + 512],
                                start=st, stop=sp,
                            )
                    nc.vector.tensor_copy(out=qkT[:, j, :], in_=pq)
                    if with_bias:
                        nc.vector.tensor_scalar_add(
                            out=qkT[:, j, :], in0=qkT[:, j, :],
                            scalar1=bq_sb[:, j:j + 1],
                        )

            # ---- phase 2: attention, one head pair at a time -----------
            with tc.tile_pool(name="ps_sc", bufs=2, space="PSUM") as ps_sc, \
                 tc.tile_pool(name="ps_ctx", bufs=4, space="PSUM") as ps_ctx:
                for p in range(NPAIR):
                    eA, eB = [], []
                    for c in range(SC):
                        psA = ps_sc.tile([NP, S], F32, tag="sc")
                        psB = ps_sc.tile([NP, S], F32, tag="sc")
                        for n in (0, 512):
                            nc.tensor.matmul(
                                psA[:, n:n + 512],
                                lhsT=qkT[0:64, NPAIR + p, c * NP:(c + 1) * NP],
                                rhs=qkT[0:64, p, n:n + 512],
                                start=True, stop=True, tile_position=(0, 0),
                            )
                            nc.tensor.matmul(
                                psB[:, n:n + 512],
                                lhsT=qkT[64:128, NPAIR + p, c * NP:(c + 1) * NP],
                                rhs=qkT[64:128, p, n:n + 512],
                                start=True, stop=True, tile_position=(64, 0),
                            )
                        tA = exps.tile([NP, S], BF16, tag="exp")
                        tB = exps.tile([NP, S], BF16, tag="exp")
                        nc.scalar.activation(tA, psA, EXP, bias=kb_sb[:, c:c + 1])
                        nc.scalar.activation(tB, psB, EXP, bias=kb_sb[:, c:c + 1])
                        eA.append(tA)
                        eB.append(tB)

                    for hi, elist in ((0, eA), (1, eB)):
                        h = 2 * p + hi
                        for half in (0, 1):
                            n0 = half * 512
                            pc = ps_ctx.tile([NP, 512], F32, tag="ctx")
                            for c in range(SC):
                                nc.tensor.matmul(
                                    pc[0:65, :],
                                    lhsT=vsb[:, c, h * 65:(h + 1) * 65],
                                    rhs=elist[c][:, n0:n0 + 512],
                                    start=(c == 0), stop=(c == SC - 1),
                                )
                            # reciprocal of the denominator row (psum row 64),
                            # staged to partition 0 via SBUF DMA: the gpsimd
                            # partition_broadcast reads through Q7 core 0,
                            # which only sees the low partitions.
                            rr = norm_pool.tile([NP, 512], F32, tag="rr")
                            nc.vector.reciprocal(out=rr[64:65, :], in_=pc[64:65, :])
                            r0 = norm_pool.tile([NP, 512], F32, tag="r0")
                            nc.sync.dma_start(out=r0[0:1, :], in_=rr[64:65, :])
                            nbc = norm_pool.tile([NP, 512], F32, tag="nbc")
                            nc.gpsimd.partition_broadcast(nbc[0:64, :], r0[0:1, :])
                            if hi == 0:
                                dst = ctxT[0:64, p, n0:n0 + 512]
                                nc.vector.tensor_mul(
                                    out=dst, in0=pc[0:64, :], in1=nbc[0:64, :],
                                )
                                if with_bias:
                                    nc.vector.tensor_scalar_add(
                                        out=dst, in0=dst,
                                        scalar1=bvcol[0:64, h:h + 1],
                                    )
                            else:
                                # DVE lanes are partition-locked; multiply at
                                # partitions 0..63 and DMA-shift to 64..127.
                                tmp = norm_pool.tile([NP, 512], BF16, tag="shift")
                                nc.vector.tensor_mul(
                                    out=tmp[0:64, :], in0=pc[0:64, :], in1=nbc[0:64, :],
                                )
                                if with_bias:
                                    nc.vector.tensor_scalar_add(
                                        out=tmp[0:64, :], in0=tmp[0:64, :],
                                        scalar1=bvcol[0:64, h:h + 1],
                                    )
                                nc.sync.dma_start(
                                    out=ctxT[64:128, p, n0:n0 + 512], in_=tmp[0:64, :],
                                )

            # ---- phase 3: output projection ----------------------------
            with tc.tile_pool(name="ps_out", bufs=3, space="PSUM") as ps_out:
                for m in range(SC):
                    po = ps_out.tile([NP, E], F32, tag="po")
                    for j in range(EC):
                        st, sp = (j == 0), (j == EC - 1)
                        nc.tensor.matmul(
                            po[:, 0:512],
                            lhsT=ctxT[:, j, m * NP:(m + 1) * NP],
                            rhs=wo_sb[:, j, 0:512],
                            start=st, stop=sp,
                        )
                        nc.tensor.matmul(
                            po[:, 512:768],
                            lhsT=ctxT[:, j, m * NP:(m + 1) * NP],
                            rhs=wo_sb[:, j, 512:768],
                            start=st, stop=sp,
                        )
                    osb = outp.tile([NP, E], F32, tag="osb")
                    nc.vector.tensor_scalar_mul(osb, po, qm_sb[:, m:m + 1])
                    if with_bias:
                        nc.vector.tensor_add(osb, osb, bo_bc)
                    nc.sync.dma_start(out=out[m * NP:(m + 1) * NP, :], in_=osb)


def build_nc(with_bias=True, repeat=1):
    nc = bacc.Bacc()
    xt = nc.dram_tensor("xt", [E, S], BF16, kind="ExternalInput")
    wqk = nc.dram_tensor("wqkvt", [E, 3 * E], BF16, kind="ExternalInput")
    bqk = nc.dram_tensor("bqkv", [3 * E], F32, kind="ExternalInput")
    wot = nc.dram_tensor("wot", [E, E], BF16, kind="ExternalInput")
    bo = nc.dram_tensor("bo", [E], F32, kind="ExternalInput")
    kb = nc.dram_tensor("kbias", [S], F32, kind="ExternalInput")
    qm = nc.dram_tensor("qmask", [S], F32, kind="ExternalInput")
    out = nc.dram_tensor("out", [S, E], F32, kind="ExternalOutput")
    with tile.TileContext(nc) as tc:
        _body(tc, xt, wqk, bqk, wot, bo, kb, qm, out, with_bias, repeat)
    nc.compile()
    return nc


def prep_in_maps(x, key_padding_mask, Wqkv_w, Wqkv_b, out_w, out_b):
    bf16 = ml_dtypes.bfloat16
    x = np.asarray(x, np.float32)
    mask = np.asarray(key_padding_mask).astype(bool)
    scale = 1.0 / np.sqrt(np.float32(D))

    wqkvT = np.asarray(Wqkv_w, np.float32).T.copy()      # (E, 3E), e-major
    wqkvT[:, :E] *= scale                                # fold 1/sqrt(d) into Wq
    bqkv = np.asarray(Wqkv_b, np.float32).copy()
    bqkv[:E] *= scale
    wotT = np.asarray(out_w, np.float32).T.copy()        # (E, E), e-major

    wqkvT = np.ascontiguousarray(wqkvT).astype(bf16)
    wotT = np.ascontiguousarray(wotT).astype(bf16)
    bo_ = np.asarray(out_b, np.float32)

    in_maps = []
    for i in range(B):
        xti = np.ascontiguousarray(x[i].T).astype(bf16)  # (E, S)
        kbias = np.where(mask[i], 0.0, MASK_NEG).astype(np.float32)
        qmask = mask[i].astype(np.float32)
        in_maps.append(
            {
                "xt": xti,
                "wqkvt": wqkvT,
                "bqkv": bqkv,
                "wot": wotT,
                "bo": bo_,
                "kbias": kbias,
                "qmask": qmask,
            }
        )
    return in_maps


_NC_CACHE = {}


def _get_nc(with_bias=True):
    if with_bias not in _NC_CACHE:
        _NC_CACHE[with_bias] = build_nc(with_bias)
    return _NC_CACHE[with_bias]


def kernel(x, key_padding_mask, Wqkv_w, Wqkv_b, out_w, out_b):
    in_maps = prep_in_maps(x, key_padding_mask, Wqkv_w, Wqkv_b, out_w, out_b)
    with_bias = bool(np.any(np.asarray(Wqkv_b) != 0) or np.any(np.asarray(out_b) != 0))
    nc = _get_nc(with_bias)
    res = run_bass_kernel_spmd(nc, in_maps, core_ids=list(range(B)))
    out = np.stack([res.results[i]["out"] for i in range(B)], axis=0)
    return out.astype(np.float32)


if __name__ == "__main__":
    nc = build_nc()
    print("build ok")


# revision 12
# speedup vs baseline: 34.8616x; 34.8616x over previous
"""BERT self-attention (flash-style) Trainium2 Bass kernel.

Full inputs -> full output. Shards data-parallel over batch: batch element i
runs on NeuronCore i (B == 8 == n_cores), no collectives.

Host-side prep (cheap numpy): transpose x / Wqkv / out_w into the e-major
layouts the TensorE needs (lhsT/rhs must both be contraction-major), fold the
1/sqrt(d) scale into the q block of Wqkv, turn the key-padding mask into an
additive exp bias (0 / -30000) and the query mask into a 0/1 multiplier.

On-chip per core (S=1024, E=768, H=12, D=64):
  phase 1: v = x @ Wv.T (natural layout, ones column per head appended)
           qT,kT = (Wq|k @ x.T)  (head dims on partitions)
  phase 2: per head pair (2p, 2p+1):
           scoresT[sk,sq] = kT.T-free matmul, 2 heads concurrently via 64-row
           PE tiling; expT = Exp(scoresT + key_bias) on ScalarE (no max
           subtraction -- scores are O(1) by construction); ctxT[d,sq]
           accumulated in PSUM with denominator from the ones column;
           normalize via VectorE reciprocal + GpSimd partition broadcast.
  phase 3: out = ctxT.T @ out_w.T, query mask as per-partition scalar, + bias.
"""

import sys

if "/opt/trn_rl_repo" not in sys.path:
    sys.path.insert(0, "/opt/trn_rl_repo")

import numpy as np
import ml_dtypes

import concourse.bass as bass
import concourse.bacc as bacc
import concourse.tile as tile
from concourse import mybir
from concourse.bass_utils import run_bass_kernel_spmd

B, S, E, H = 8, 1024, 768, 12
D = E // H            # 64
NP = 128              # SBUF/PSUM partitions
EC = E // NP          # 6 e-chunks (contraction chunks)
FC = 3 * E // NP      # 18 f-chunks of the fused qkv output
SC = S // NP          # 8 sequence chunks
NPAIR = H // 2        # 6 head pairs
BF16 = mybir.dt.bfloat16
F32 = mybir.dt.float32
EXP = mybir.ActivationFunctionType.Exp
MASK_NEG = -30000.0
_TIMING_NO_BCAST = False   # exp(x + MASK_NEG) == 0 for any realistic score x


def _body(tc, xt, wqk, bqk, wot, bo, kb, qm, out, with_bias, repeat=1):
    nc = tc.nc

    with tc.tile_pool(name="const", bufs=1) as const:
        # ---- persistent SBUF state -------------------------------------
        xt_sb = const.tile([NP, EC, S], BF16)      # x.T   [e, s]
        nc.sync.dma_start(out=xt_sb, in_=xt.rearrange("(c p) s -> p c s", p=NP))
        wq_sb = const.tile([NP, EC, 3 * E], BF16)  # Wqkv.T [e, f]
        nc.sync.dma_start(out=wq_sb, in_=wqk.rearrange("(c p) f -> p c f", p=NP))
        wo_sb = const.tile([NP, EC, E], BF16)      # out_w.T [e, eout]
        nc.sync.dma_start(out=wo_sb, in_=wot.rearrange("(c p) f -> p c f", p=NP))
        if with_bias:
            bq_sb = const.tile([NP, FC], F32)      # qkv bias, col j = f-chunk j
            nc.sync.dma_start(out=bq_sb, in_=bqk.rearrange("(c p) -> p c", p=NP))
            # v bias per head-dim column: col h = bias[2E + 64h + p], p in 0..63
            bvcol = const.tile([NP, H], F32)
            nc.sync.dma_start(
                out=bvcol[0:64, :],
                in_=bass.AP(tensor=bqk, offset=2 * E, ap=[[1, 64], [64, H]]),
            )
        kb_sb = const.tile([NP, SC], F32)          # key mask bias, col c = s-chunk c
        nc.sync.dma_start(out=kb_sb, in_=kb.rearrange("(c p) -> p c", p=NP))
        qm_sb = const.tile([NP, SC], F32)          # query mask 0/1, col m = s-chunk m
        nc.sync.dma_start(out=qm_sb, in_=qm.rearrange("(c p) -> p c", p=NP))
        if with_bias:
            bo_bc = const.tile([NP, E], F32)       # out bias broadcast
            nc.sync.dma_start(
                out=bo_bc, in_=bass.AP(tensor=bo, offset=0, ap=[[0, NP], [1, E]])
            )

        for _rep in range(repeat):
            _compute(tc, nc, with_bias,
                     xt_sb, wq_sb, wo_sb, kb_sb, qm_sb, out,
                     bq_sb if with_bias else None,
                     bvcol if with_bias else None,
                     bo_bc if with_bias else None)


def _compute(tc, nc, with_bias, xt_sb, wq_sb, wo_sb, kb_sb, qm_sb, out,
             bq_sb, bvcol, bo_bc):
    with tc.tile_pool(name="work", bufs=1) as work:
        # qT/kT: [128, j, s] bf16; partition = f within chunk. j=0..5 q pairs
        # (heads 2j,2j+1 at partitions 0-63 / 64-127), j=6..11 k pairs.
        qkT = work.tile([NP, H, S], BF16)
        # v (+ per-head denominator column): s-chunk m on partitions.
        # Head slot of 65 columns: [v(64), ones] -- the ones column makes the
        # ctx matmul emit the softmax denominator as psum row 64 for free.
        vsb = work.tile([NP, SC, H * 65], BF16)
        # ctx.T: pair j -> partitions 0:64 head 2j, 64:128 head 2j+1; e-chunk j.
        ctxT = work.tile([NP, EC, S], BF16)

        with tc.tile_pool(name="norm", bufs=4) as norm_pool, \
             tc.tile_pool(name="exps", bufs=18) as exps, \
             tc.tile_pool(name="osb", bufs=3) as outp:

            # ---- phase 1: fused qkv projection -------------------------
            with tc.tile_pool(name="ps_qkv", bufs=3, space="PSUM") as ps_qkv:
                # v natural: [s-chunk, f] so it can be ctx lhsT later
                for m in range(SC):
                    pv = ps_qkv.tile([NP, E], F32, tag="ps")
                    for k in range(EC):
                        st, sp = (k == 0), (k == EC - 1)
                        nc.tensor.matmul(
                            pv[:, 0:512],
                            lhsT=xt_sb[:, k, m * NP:(m + 1) * NP],
                            rhs=wq_sb[:, k, 2 * E:2 * E + 512],
                            start=st, stop=sp,
                        )
                        nc.tensor.matmul(
                            pv[:, 512:768],
                            lhsT=xt_sb[:, k, m * NP:(m + 1) * NP],
                            rhs=wq_sb[:, k, 2 * E + 512:3 * E],
                            start=st, stop=sp,
                        )
                    # per-head slot [v(64), ones]: the ones column makes the
                    # ctx matmul emit the softmax denominator as psum row 64
                    v_m = vsb[:, m, :].rearrange("p (h t) -> p h t", t=65)
                    nc.vector.memset(v_m[:, :, 64:65], 1.0)
                    pv_h = pv.rearrange("p (h d) -> p h d", d=D)
                    nc.vector.tensor_copy(out=v_m[:, :, 0:64], in_=pv_h)

                # qT, kT: f-chunk j on partitions, s free
                for j in range(H):
                    pq = ps_qkv.tile([NP, S], F32, tag="ps")
                    for k in range(EC):
                        st, sp = (k == 0), (k == EC - 1)
                        for n in (0, 512):
                            nc.tensor.matmul(
                                pq[:, n:n + 512],
                                lhsT=wq_sb[:, k, j * NP:(j + 1) * NP],
                                rhs=xt_sb[:, k, n:n + 512],
                                start=st, stop=sp,
                            )
                    nc.vector.tensor_copy(out=qkT[:, j, :], in_=pq)
                    if with_bias:
                        nc.vector.tensor_scalar_add(
                            out=qkT[:, j, :], in0=qkT[:, j, :],
                            scalar1=bq_sb[:, j:j + 1],
                        )

            # ---- phase 2: attention, one head pair at a time -----------
            with tc.tile_pool(name="ps_sc", bufs=2, space="PSUM") as ps_sc, \
                 tc.tile_pool(name="ps_ctx", bufs=4, space="PSUM") as ps_ctx:
                for p in range(NPAIR):
                    eA, eB = [], []
                    for c in range(SC):
                        psA = ps_sc.tile([NP, S], F32, tag="sc")
                        psB = ps_sc.tile([NP, S], F32, tag="sc")
                        for n in (0, 512):
                            nc.tensor.matmul(
                                psA[:, n:n + 512],
                                lhsT=qkT[0:64, NPAIR + p, c * NP:(c + 1) * NP],
                                rhs=qkT[0:64, p, n:n + 512],
                                start=True, stop=True, tile_position=(0, 0),
                            )
                            nc.tensor.matmul(
                                psB[:, n:n + 512],
                                lhsT=qkT[64:128, NPAIR + p, c * NP:(c + 1) * NP],
                                rhs=qkT[64:128, p, n:n + 512],
                                start=True, stop=True, tile_position=(64, 0),
                            )
                        tA = exps.tile([NP, S], BF16, tag="exp")
                        tB = exps.tile([NP, S], BF16, tag="exp")
                        nc.scalar.activation(tA, psA, EXP, bias=kb_sb[:, c:c + 1])
                        nc.scalar.activation(tB, psB, EXP, bias=kb_sb[:, c:c + 1])
                        eA.append(tA)
                        eB.append(tB)

                    for hi, elist in ((0, eA), (1, eB)):
                        h = 2 * p + hi
                        for half in (0, 1):
                            n0 = half * 512
                            pc = ps_ctx.tile([NP, 512], F32, tag="ctx")
                            for c in range(SC):
                                nc.tensor.matmul(
                                    pc[0:65, :],
                                    lhsT=vsb[:, c, h * 65:(h + 1) * 65],
                                    rhs=elist[c][:, n0:n0 + 512],
                                    start=(c == 0), stop=(c == SC - 1),
                                )
                            # reciprocal of the denominator row (psum row 64),
                            # staged to partition 0 via SBUF DMA: the gpsimd
                            # partition_broadcast reads through Q7 core 0,
                            # which only sees the low partitions.
                            rr = norm_pool.tile([NP, 512], F32, tag="rr")
                            nc.vector.reciprocal(out=rr[64:65, :], in_=pc[64:65, :])
                            r0 = norm_pool.tile([NP, 512], F32, tag="r0")
                            nc.sync.dma_start(out=r0[0:1, :], in_=rr[64:65, :])
                            nbc = norm_pool.tile([NP, 512], F32, tag="nbc")
                            if _TIMING_NO_BCAST:
                                nc.vector.tensor_copy(out=nbc[0:64, :], in_=rr[0:64, :])
                            else:
                                nc.gpsimd.partition_broadcast(nbc[0:64, :], r0[0:1, :])
                            if hi == 0:
                                dst = ctxT[0:64, p, n0:n0 + 512]
                                nc.vector.tensor_mul(
                                    out=dst, in0=pc[0:64, :], in1=nbc[0:64, :],
                                )
                                if with_bias:
                                    nc.vector.tensor_scalar_add(
                                        out=dst, in0=dst,
                                        scalar1=bvcol[0:64, h:h + 1],
                                    )
                            else:
                                # DVE lanes are partition-locked; multiply at
                                # partitions 0..63 and DMA-shift to 64..127.
                                tmp = norm_pool.tile([NP, 512], BF16, tag="shift")
                                nc.vector.tensor_mul(
                                    out=tmp[0:64, :], in0=pc[0:64, :], in1=nbc[0:64, :],
                                )
                                if with_bias:
                                    nc.vector.tensor_scalar_add(
                                        out=tmp[0:64, :], in0=tmp[0:64, :],
                                        scalar1=bvcol[0:64, h:h + 1],
                                    )
                                nc.sync.dma_start(
                                    out=ctxT[64:128, p, n0:n0 + 512], in_=tmp[0:64, :],
                                )

            # ---- phase 3: output projection ----------------------------
            with tc.tile_pool(name="ps_out", bufs=3, space="PSUM") as ps_out:
                for m in range(SC):
                    po = ps_out.tile([NP, E], F32, tag="po")
                    for j in range(EC):
                        st, sp = (j == 0), (j == EC - 1)
                        nc.tensor.matmul(
                            po[:, 0:512],
                            lhsT=ctxT[:, j, m * NP:(m + 1) * NP],
                            rhs=wo_sb[:, j, 0:512],
                            start=st, stop=sp,
                        )
                        nc.tensor.matmul(
                            po[:, 512:768],
                            lhsT=ctxT[:, j, m * NP:(m + 1) * NP],
                            rhs=wo_sb[:, j, 512:768],
                            start=st, stop=sp,
                        )
                    osb = outp.tile([NP, E], F32, tag="osb")
                    nc.vector.tensor_scalar_mul(osb, po, qm_sb[:, m:m + 1])
                    if with_bias:
                        nc.vector.tensor_add(osb, osb, bo_bc)
                    nc.sync.dma_start(out=out[m * NP:(m + 1) * NP, :], in_=osb)


def build_nc(with_bias=True, repeat=1):
    nc = bacc.Bacc()
    xt = nc.dram_tensor("xt", [E, S], BF16, kind="ExternalInput")
    wqk = nc.dram_tensor("wqkvt", [E, 3 * E], BF16, kind="ExternalInput")
    bqk = nc.dram_tensor("bqkv", [3 * E], F32, kind="ExternalInput")
    wot = nc.dram_tensor("wot", [E, E], BF16, kind="ExternalInput")
    bo = nc.dram_tensor("bo", [E], F32, kind="ExternalInput")
    kb = nc.dram_tensor("kbias", [S], F32, kind="ExternalInput")
    qm = nc.dram_tensor("qmask", [S], F32, kind="ExternalInput")
    out = nc.dram_tensor("out", [S, E], F32, kind="ExternalOutput")
    with tile.TileContext(nc) as tc:
        _body(tc, xt, wqk, bqk, wot, bo, kb, qm, out, with_bias, repeat)
    nc.compile()
    return nc


def prep_in_maps(x, key_padding_mask, Wqkv_w, Wqkv_b, out_w, out_b):
    bf16 = ml_dtypes.bfloat16
    x = np.asarray(x, np.float32)
    mask = np.asarray(key_padding_mask).astype(bool)
    scale = 1.0 / np.sqrt(np.float32(D))

    wqkvT = np.asarray(Wqkv_w, np.float32).T.copy()      # (E, 3E), e-major
    wqkvT[:, :E] *= scale                                # fold 1/sqrt(d) into Wq
    bqkv = np.asarray(Wqkv_b, np.float32).copy()
    bqkv[:E] *= scale
    wotT = np.asarray(out_w, np.float32).T.copy()        # (E, E), e-major

    wqkvT = np.ascontiguousarray(wqkvT).astype(bf16)
    wotT = np.ascontiguousarray(wotT).astype(bf16)
    bo_ = np.asarray(out_b, np.float32)

    in_maps = []
    for i in range(B):
        xti = np.ascontiguousarray(x[i].T).astype(bf16)  # (E, S)
        kbias = np.where(mask[i], 0.0, MASK_NEG).astype(np.float32)
        qmask = mask[i].astype(np.float32)
        in_maps.append(
            {
                "xt": xti,
                "wqkvt": wqkvT,
                "bqkv": bqkv,
                "wot": wotT,
                "bo": bo_,
                "kbias": kbias,
                "qmask": qmask,
            }
        )
    return in_maps


_NC_CACHE = {}


def _get_nc(with_bias=True):
    if with_bias not in _NC_CACHE:
        _NC_CACHE[with_bias] = build_nc(with_bias)
    return _NC_CACHE[with_bias]


def kernel(x, key_padding_mask, Wqkv_w, Wqkv_b, out_w, out_b):
    in_maps = prep_in_maps(x, key_padding_mask, Wqkv_w, Wqkv_b, out_w, out_b)
    with_bias = bool(np.any(np.asarray(Wqkv_b) != 0) or np.any(np.asarray(out_b) != 0))
    nc = _get_nc(with_bias)
    res = run_bass_kernel_spmd(nc, in_maps, core_ids=list(range(B)))
    out = np.stack([res.results[i]["out"] for i in range(B)], axis=0)
    return out.astype(np.float32)


if __name__ == "__main__":
    nc = build_nc()
    print("build ok")
